# revision 11
# baseline (speedup 1.0000x reference)
"""BiMPM Trainium2 Bass kernel — pure data parallel over batch (B=32 -> 4/core).

Per-core layouts (B_l=4, stack S=8 rows per step = [p:b0..3, h:b0..3]):
- token/row order: r = t*8 + s, s = seq*4 + b (seq0 = q1 = "p", seq1 = q2 = "h")
- xg projections: (128 = 16t x 8s, m=4, 1024) bf16 per dir
- gates psum: fw rows [0:8], bw rows [32:40] (col-tiled bf16 matmuls)
- scan state c/h: (64p, 256) f32, rows [0:8] fw / [32:40] bw
- conT (ctx outputs, hd-major): (128 = hd%128, 2c, 8s, 64t) f32r per dir
- mvT (match features): 2 tiles (128, 512 = 8t*64... cols r = 8t+s) f32r,
  feature rows at 32-aligned slots [full@0, max@32, am@64, ax@96, ones@116]
"""
import numpy as np
from contextlib import ExitStack

import concourse.bass as bass
import concourse.tile as tile
from concourse import bacc, mybir
from concourse.bass_utils import run_bass_kernel_spmd
from concourse.masks import make_identity

F32 = mybir.dt.float32
F32R = mybir.dt.float32r
BF16 = mybir.dt.bfloat16
I32 = mybir.dt.int32
AF = mybir.ActivationFunctionType
ALU = mybir.AluOpType
AX_X = mybir.AxisListType.X

B, T, V, D, H, L, NL = 32, 64, 50000, 300, 256, 20, 2
NCORES = 8
BL = B // NCORES
S = 2 * BL
EPS = 1e-8

_CACHE = {}


# ---------------------------------------------------------------- host prep

def _gate_reorder(w):
    i, f, g, o = np.split(w, 4, axis=0)
    return np.concatenate([i, f, o, g], axis=0)


def _prep_weights(inp):
    w = {}
    f32 = np.float32

    def ctx_wT(dir_):
        wih = _gate_reorder(np.asarray(inp[f'ctx_wih_{dir_}'], f32))
        bias = _gate_reorder(
            np.asarray(inp[f'ctx_bih_{dir_}'] + inp[f'ctx_bhh_{dir_}'],
                       f32)[:, None]).T
        return np.ascontiguousarray(np.concatenate([wih.T, bias], 0), f32)

    def whhT(pfx, dir_):
        whh = _gate_reorder(np.asarray(inp[f'{pfx}_whh_{dir_}'], f32))
        return np.ascontiguousarray(
            whh.T.reshape(2, 128, 1024).transpose(1, 0, 2), f32)

    w['wihT_f'], w['wihT_b'] = ctx_wT('f'), ctx_wT('b')
    w['whhT_f'], w['whhT_b'] = whhT('ctx', 'f'), whhT('ctx', 'b')
    w['awhhT_f'], w['awhhT_b'] = whhT('agg', 'f'), whhT('agg', 'b')

    def agg_wT(dir_):
        wih = _gate_reorder(np.asarray(inp[f'agg_wih_{dir_}'], f32))
        bias = _gate_reorder(
            np.asarray(inp[f'agg_bih_{dir_}'] + inp[f'agg_bhh_{dir_}'],
                       f32)[:, None]).T
        out = np.zeros((256, 1024), f32)
        for d in range(2):
            for ty in range(4):
                src = wih[:, d * 80 + ty * 20: d * 80 + ty * 20 + 20]
                out[d * 128 + 32 * ty: d * 128 + 32 * ty + 20] = src.T
        out[116] = bias[0]
        return np.ascontiguousarray(
            out.reshape(2, 128, 1024).transpose(1, 0, 2), f32)

    w['aggwT_f'], w['aggwT_b'] = agg_wT('f'), agg_wT('b')

    w2 = np.asarray(inp['mp_w'], f32) ** 2
    w2t = np.zeros((128, 2, 8, 20), f32)
    for d in range(2):
        for ty in range(4):
            src = w2[2 * ty + d]
            for c in range(2):
                w2t[:, c, d * 4 + ty, :] = src[:, c * 128:(c + 1) * 128].T
    w['w2T'] = np.ascontiguousarray(w2t)

    fc1 = np.asarray(inp['fc1_w'], f32)
    w['fc1T'] = np.ascontiguousarray(
        fc1.T.reshape(8, 128, 512).transpose(1, 0, 2))
    w['fc1b'] = np.ascontiguousarray(
        np.broadcast_to(np.asarray(inp['fc1_b'], f32), (BL, 512)))
    fc2 = np.asarray(inp['fc2_w'], f32)
    w['fc2T'] = np.ascontiguousarray(
        fc2.T.reshape(4, 128, 2).transpose(1, 0, 2))
    w['fc2b'] = np.ascontiguousarray(
        np.broadcast_to(np.asarray(inp['fc2_b'], f32), (BL, 2)))
    w['word_emb'] = np.ascontiguousarray(np.asarray(inp['word_emb'], f32))
    return w


def _prep_tokens(q1, q2, core):
    q1c = np.asarray(q1[core * BL:(core + 1) * BL]).astype(np.int64)
    q2c = np.asarray(q2[core * BL:(core + 1) * BL]).astype(np.int64)
    tok = np.zeros((T * S,), np.int32)
    for seq, q in ((0, q1c), (1, q2c)):
        for b in range(BL):
            tok[np.arange(T) * S + seq * BL + b] = q[b]
    return np.ascontiguousarray(tok.reshape(4, 128))


# ---------------------------------------------------------------- build

def build_nc(debug=False):
    nc = bacc.Bacc("TRN2", target_bir_lowering=False, debug=False,
                   enable_asserts=True, num_devices=NCORES)
    dt = nc.dram_tensor
    dr = {}
    dr['tokp'] = dt("tokp", [4, 128], I32, kind="ExternalInput").ap()
    dr['word_emb'] = dt("word_emb", [V, D], F32, kind="ExternalInput").ap()
    for n, shp in [('wihT_f', [301, 1024]), ('wihT_b', [301, 1024]),
                   ('whhT_f', [128, 2, 1024]), ('whhT_b', [128, 2, 1024]),
                   ('awhhT_f', [128, 2, 1024]), ('awhhT_b', [128, 2, 1024]),
                   ('aggwT_f', [128, 2, 1024]), ('aggwT_b', [128, 2, 1024]),
                   ('w2T', [128, 2, 8, 20]), ('fc1T', [128, 8, 512]),
                   ('fc1b', [BL, 512]), ('fc2T', [128, 4, 2]),
                   ('fc2b', [BL, 2])]:
        dr[n] = dt(n, shp, F32, kind="ExternalInput").ap()
    y = dt("y", [BL, NL], F32, kind="ExternalOutput").ap()
    dbg = {}
    if debug:
        dbg['conT_f'] = dt("dbg_conT_f", [128, 2, 8, 64], F32,
                           kind="ExternalOutput").ap()
        dbg['conT_b'] = dt("dbg_conT_b", [128, 2, 8, 64], F32,
                           kind="ExternalOutput").ap()
        dbg['mvT0'] = dt("dbg_mvT0", [128, 512], F32,
                         kind="ExternalOutput").ap()
        dbg['mvT1'] = dt("dbg_mvT1", [128, 512], F32,
                         kind="ExternalOutput").ap()
        dbg['xT'] = dt("dbg_xT", [128, 2, 40], F32,
                       kind="ExternalOutput").ap()

    with tile.TileContext(nc) as tc, ExitStack() as ctx:
        _body(nc, tc, ctx, dr, y, dbg)
    nc.compile()
    return nc


def _body(nc, tc, ctx, dr, y, dbg):
    perm = ctx.enter_context(tc.tile_pool(name="perm", bufs=1))

    idf = perm.tile([128, 128], F32, name="idf")
    make_identity(nc, idf[:])
    idb = perm.tile([128, 128], BF16, name="idb")
    nc.vector.tensor_copy(idb[:], idf[:])
    selb = idb.rearrange("k (tl s) -> k tl s", s=8)
    idr4 = perm.tile([4, 4], F32R, name="idr4")
    nc.vector.tensor_copy(idr4[:], idf[0:4, 0:4])
    idr = perm.tile([128, 128], F32R, name="idr")
    nc.gpsimd.tensor_copy(idr[:], idf[:])

    def conv(src, dtype, name, engine=None, pool=None):
        t = (pool or perm).tile(list(src.shape), dtype, name=f"C_{name}")
        eng = engine or nc.vector
        if eng is nc.scalar:
            eng.activation(t[:], src[:], AF.Copy)
        else:
            eng.tensor_copy(t[:], src[:])
        return t

    wihT, whhTb, awhhTb, aggwT = {}, {}, {}, {}
    w2Tf = perm.tile([128, 2, 8, 20], F32, name="w2Tf")
    nc.sync.dma_start(w2Tf[:], dr['w2T'][:])
    w2T = conv(w2Tf, F32R, "w2T")
    with tc.tile_pool(name="loadp", bufs=1) as loadp:
        def load_f32(name, shp):
            t = loadp.tile(shp, F32, name=f"L_{name}", tag="raw8k")
            nc.sync.dma_start(t[:], dr[name][:])
            return t

        for d in 'fb':
            raw = loadp.tile([128, 3, 1024], F32, name=f"wihraw_{d}",
                             tag="raw12k")
            for c in range(3):
                kc = min(128, 301 - 128 * c)
                nc.sync.dma_start(raw[0:kc, c, :],
                                  dr[f'wihT_{d}'][128 * c:128 * c + kc, :])
            wihT[d] = conv(raw, F32R, f"wihT_{d}", nc.scalar)
            whhTb[d] = conv(load_f32(f'whhT_{d}', [128, 2, 1024]), BF16,
                            f"whh_{d}")
            awhhTb[d] = conv(load_f32(f'awhhT_{d}', [128, 2, 1024]), BF16,
                             f"awhh_{d}", nc.gpsimd)
            aggwT[d] = conv(load_f32(f'aggwT_{d}', [128, 2, 1024]), F32R,
                            f"aggw_{d}", nc.scalar)

    idx_sb = perm.tile([128, 4], I32, name="idx_sb")
    nc.sync.dma_start(idx_sb[:], dr['tokp'].rearrange("m p -> p m"))

    # ---------------- embedding gather + ctx input projection
    xg = {'f': perm.tile([128, 4, 1024], BF16, name="xg_f"),
          'b': perm.tile([128, 4, 1024], BF16, name="xg_b")}
    with tc.tile_pool(name="embp", bufs=2) as embp, \
         tc.tile_pool(name="epsum", bufs=2, space="PSUM") as epsum:
        for m in range(4):
            emb = embp.tile([128, 304], F32, name=f"emb_{m}", tag="emb")
            nc.gpsimd.indirect_dma_start(
                out=emb[:, 0:300], out_offset=None, in_=dr['word_emb'][:],
                in_offset=bass.IndirectOffsetOnAxis(ap=idx_sb[:, m:m + 1],
                                                    axis=0))
            nc.vector.memset(emb[:, 300:301], 1.0)
            embT = embp.tile([128, 3, 128], F32R, name=f"embT_{m}", tag="embT")
            for c in range(3):
                kc = min(128, 301 - 128 * c)
                tp = epsum.tile([128, 128], F32, name=f"etp_{m}_{c}",
                                tag="etp")
                nc.tensor.transpose(tp[0:kc, :], emb[:, 128 * c:128 * c + kc],
                                    idf[:])
                nc.scalar.activation(embT[0:kc, c, :], tp[0:kc, :], AF.Copy)
            for di, d in enumerate('fb'):
                ps = epsum.tile([128, 1024], F32, name=f"xps_{m}_{d}",
                                tag="xps")
                for nh in range(2):
                    for c in range(3):
                        kc = min(128, 301 - 128 * c)
                        nc.tensor.matmul(
                            ps[:, 512 * nh:512 * (nh + 1)],
                            embT[0:kc, c, :],
                            wihT[d][0:kc, c, 512 * nh:512 * (nh + 1)],
                            start=(c == 0), stop=(c == 2))
                if di == 0:
                    nc.vector.tensor_copy(xg[d][:, m, :], ps[:])
                else:
                    nc.scalar.activation(xg[d][:, m, :], ps[:], AF.Copy)

    # ---------------- scan layer (shared ctx/agg)
    def scan_layer(xgd, whh_d, conT_out, xT_out, lname):
        sp = ctx2.enter_context(tc.tile_pool(name=f"sp_{lname}", bufs=3))
        pp = ctx2.enter_context(tc.tile_pool(name=f"pp_{lname}", bufs=2,
                                             space="PSUM"))
        cp = ctx2.enter_context(tc.tile_pool(name=f"cp_{lname}", bufs=1))
        c_sb = cp.tile([64, 256], F32, name=f"c_{lname}")
        hT_prev = None
        for tau in range(T):
            ts_ = {'f': tau, 'b': T - 1 - tau}
            gps = pp.tile([64, 1024], F32, name=f"g_{lname}_{tau}", tag="gps")
            for di, d in enumerate('fb'):
                t = ts_[d]
                row = 32 * di
                pos = (0, row)
                for nh in range(2):
                    if hT_prev is not None:
                        for c in range(2):
                            nc.tensor.matmul(
                                gps[row:row + S, 512 * nh:512 * (nh + 1)],
                                hT_prev[:, c, row:row + S],
                                whh_d[d][:, c, 512 * nh:512 * (nh + 1)],
                                start=(c == 0), stop=False, tile_position=pos)
                    nc.tensor.matmul(
                        gps[row:row + S, 512 * nh:512 * (nh + 1)],
                        selb[:, t % 16, :],
                        xgd[d][:, t // 16, 512 * nh:512 * (nh + 1)],
                        start=(hT_prev is None), stop=True, tile_position=pos)
            sig = sp.tile([64, 768], F32, name=f"si_{lname}_{tau}", tag="sig")
            nc.scalar.activation(sig[0:40, :], gps[0:40, 0:768], AF.Sigmoid)
            tg = sp.tile([64, 256], F32, name=f"tg_{lname}_{tau}", tag="tg")
            nc.scalar.activation(tg[0:40, :], gps[0:40, 768:1024], AF.Tanh)
            if hT_prev is None:
                nc.gpsimd.tensor_tensor(out=c_sb[0:40, :],
                                        in0=sig[0:40, 0:256],
                                        in1=tg[0:40, :], op=ALU.mult)
            else:
                t1 = sp.tile([64, 256], F32, name=f"t1_{lname}_{tau}",
                             tag="t1")
                nc.vector.tensor_tensor(out=t1[0:40, :],
                                        in0=sig[0:40, 256:512],
                                        in1=c_sb[0:40, :], op=ALU.mult)
                t2 = sp.tile([64, 256], F32, name=f"t2_{lname}_{tau}",
                             tag="t2")
                nc.gpsimd.tensor_tensor(out=t2[0:40, :],
                                        in0=sig[0:40, 0:256],
                                        in1=tg[0:40, :], op=ALU.mult)
                nc.vector.tensor_tensor(out=c_sb[0:40, :], in0=t1[0:40, :],
                                        in1=t2[0:40, :], op=ALU.add)
            th = sp.tile([64, 256], F32, name=f"th_{lname}_{tau}", tag="th")
            nc.scalar.activation(th[0:40, :], c_sb[0:40, :], AF.Tanh)
            h_sb = sp.tile([64, 256], F32, name=f"h_{lname}_{tau}", tag="h")
            nc.gpsimd.tensor_tensor(out=h_sb[0:40, :], in0=sig[0:40, 512:768],
                                    in1=th[0:40, :], op=ALU.mult)
            hT_ps = pp.tile([128, 2, 40], F32, name=f"hp_{lname}_{tau}",
                            tag="hTps")
            for c in range(2):
                nc.tensor.transpose(hT_ps[:, c, :],
                                    h_sb[0:40, 128 * c:128 * (c + 1)],
                                    idf[0:40, 0:40])
            hT_bf = sp.tile([128, 2, 40], BF16, name=f"hb_{lname}_{tau}",
                            tag="hTb")
            nc.vector.tensor_copy(hT_bf[:], hT_ps[:])
            hT_prev = hT_bf
            if conT_out is not None:
                nc.vector.tensor_copy(conT_out['f'][:, :, :, ts_['f']],
                                      hT_ps[:, :, 0:8])
                nc.scalar.activation(conT_out['b'][:, :, :, ts_['b']],
                                     hT_ps[:, :, 32:40], AF.Copy)
            if xT_out is not None and tau == T - 1:
                nc.vector.tensor_copy(xT_out[:], hT_ps[:])

    conT = {'f': perm.tile([128, 2, 8, 64], F32R, name="conT_f"),
            'b': perm.tile([128, 2, 8, 64], F32R, name="conT_b")}
    with ExitStack() as ctx2:
        scan_layer(xg, whhTb, conT, None, "ctx")

    # ---------------- matching
    mvT = [perm.tile([128, 512], F32R, name="mvT0"),
           perm.tile([128, 512], F32R, name="mvT1")]
    # f32r memset unsupported; fill via ACT copy with scale=0 (+bias)
    fill_src = bass.AP(tensor=idf.tensor, offset=idf.offset,
                       ap=[idf.ap[0], [0, 512]])
    nc.scalar.activation(mvT[0][:], fill_src, AF.Copy, bias=0.0, scale=0.0)
    nc.scalar.activation(mvT[1][:], fill_src, AF.Copy, bias=0.0, scale=0.0)
    nc.scalar.activation(mvT[0][96:128, :],
                         bass.AP(tensor=idf.tensor, offset=idf.offset,
                                 ap=[[idf.ap[0][0], 32], [0, 512]]),
                         AF.Copy, bias=1.0, scale=0.0)
    _matching(nc, tc, ctx, conT, w2T, w2Tf, mvT, idf, idr)

    # ---------------- agg projection
    xga = {'f': perm.tile([128, 4, 1024], BF16, name="xga_f"),
           'b': perm.tile([128, 4, 1024], BF16, name="xga_b")}
    with tc.tile_pool(name="aggps", bufs=2, space="PSUM") as ap_ps:
        for di, d in enumerate('fb'):
            for m in range(4):
                ps = ap_ps.tile([128, 1024], F32, name=f"ap_{d}_{m}",
                                tag="aps")
                for nh in range(2):
                    for kc in range(2):
                        nc.tensor.matmul(
                            ps[:, 512 * nh:512 * (nh + 1)],
                            mvT[kc][:, 128 * m:128 * (m + 1)],
                            aggwT[d][:, kc, 512 * nh:512 * (nh + 1)],
                            start=(kc == 0), stop=(kc == 1))
                if (m + di) % 2 == 0:
                    nc.vector.tensor_copy(xga[d][:, m, :], ps[:])
                else:
                    nc.scalar.activation(xga[d][:, m, :], ps[:], AF.Copy)

    # ---------------- agg scans + fc
    xT = perm.tile([128, 2, 40], F32, name="xT_agg")
    with ExitStack() as ctx2:
        scan_layer(xga, awhhTb, None, xT, "agg")

    fcp = ctx.enter_context(tc.tile_pool(name="fcp", bufs=1))
    fps = ctx.enter_context(tc.tile_pool(name="fcps", bufs=1, space="PSUM"))
    fc1T = fcp.tile([128, 8, 512], F32R, name="fc1T")
    fc2T = fcp.tile([128, 4, 2], F32R, name="fc2T")
    fc1b = fcp.tile([BL, 512], F32R, name="fc1b")
    fc2b = fcp.tile([BL, 2], F32R, name="fc2b")
    with tc.tile_pool(name="fcload", bufs=1) as fcl:
        for nm, tgt in (('fc1T', fc1T), ('fc2T', fc2T), ('fc1b', fc1b),
                        ('fc2b', fc2b)):
            rawf = fcl.tile(list(tgt.shape), F32, name=f"rf_{nm}", tag="rawfc")
            nc.sync.dma_start(rawf[:], dr[nm][:])
            nc.vector.tensor_copy(tgt[:], rawf[:])
    xTr = fcp.tile([128, 2, 40], F32R, name="xTr")
    nc.vector.tensor_copy(xTr[:], xT[:])
    # k-chunks: [hpf c0, hpf c1, hpb c0, hpb c1, hhf c0, hhf c1, hhb c0, hhb c1]
    ksl = []
    for role0 in (0, 4):
        for base in (0, 32):
            for c in range(2):
                ksl.append(xTr[:, c, base + role0:base + role0 + BL])
    ksl = [ksl[0], ksl[1], ksl[2], ksl[3], ksl[4], ksl[5], ksl[6], ksl[7]]
    x1 = fps.tile([BL, 512], F32, name="x1")
    for kc in range(8):
        nc.tensor.matmul(x1[:], ksl[kc], fc1T[:, kc, :],
                         start=(kc == 0), stop=False)
    nc.tensor.matmul(x1[:], idr4[:], fc1b[:], start=False, stop=True)
    xt1 = fcp.tile([BL, 512], F32, name="xt1")
    nc.scalar.activation(xt1[:], x1[:], AF.Tanh)
    xt1ps = fps.tile([128, 4, BL], F32, name="xt1ps")
    for c in range(4):
        nc.tensor.transpose(xt1ps[:, c, :], xt1[:, 128 * c:128 * (c + 1)],
                            idf[0:BL, 0:BL])
    xt1T = fcp.tile([128, 4, BL], F32R, name="xt1T")
    nc.vector.tensor_copy(xt1T[:], xt1ps[:])
    yps = fps.tile([BL, NL], F32, name="yps")
    for c in range(4):
        nc.tensor.matmul(yps[:], xt1T[:, c, :], fc2T[:, c, :],
                         start=(c == 0), stop=False)
    nc.tensor.matmul(yps[:], idr4[:], fc2b[:], start=False, stop=True)
    y_sb = fcp.tile([BL, NL], F32, name="y_sb")
    nc.vector.tensor_copy(y_sb[:], yps[:])
    nc.sync.dma_start(y[:], y_sb[:])

    if dbg:
        for d in 'fb':
            cf = fcp.tile([128, 2, 8, 64], F32, name=f"dbgc_{d}")
            nc.scalar.activation(cf[:], conT[d][:], AF.Copy)
            nc.sync.dma_start(dbg[f'conT_{d}'][:], cf[:])
        for i in range(2):
            mf = fcp.tile([128, 512], F32, name=f"dbgm_{i}")
            nc.scalar.activation(mf[:], mvT[i][:], AF.Copy)
            nc.sync.dma_start(dbg[f'mvT{i}'][:], mf[:])
        nc.sync.dma_start(dbg['xT'][:], xT[:])


# ---------------------------------------------------------------- matching

def _matching(nc, tc, ctx, conT, w2T, w2Tf, mvT, idf, idr):
    FULL, MAX, AM, AX = 0, 1, 2, 3
    mp = ctx.enter_context(tc.tile_pool(name="mp", bufs=1))
    dramp = ctx.enter_context(tc.tile_pool(name="mdram", bufs=1,
                                           space="DRAM"))

    def mcol(mt, slot, ri, b):
        # (20, 64) view of mvT rows [slot:slot+20], cols 8t + ri*4 + b
        return mt[slot:slot + 20, :].rearrange("l (t s) -> l t s",
                                               s=8)[:, :, ri * BL + b]

    for di, d in enumerate('fb'):
        cT = conT[d]
        anchor_t = (T - 1) if d == 'f' else 0
        mt = mvT[di]

        with tc.tile_pool(name=f"mn_{d}", bufs=1) as mn, \
             tc.tile_pool(name=f"mnp_{d}", bufs=1, space="PSUM") as mnp:
            # squares, bf16 copy
            csq = mp.tile([128, 2, 8, 64], F32R, name=f"csq_{d}", tag="csq")
            nc.scalar.activation(csq[:], cT[:], AF.Square)
            cbf = mp.tile([128, 2, 8, 64], BF16, name=f"cbf_{d}", tag="cbf")
            nc.vector.tensor_copy(cbf[:], cT[:])

            # norms n1[l, ty, s, t] (f32, sqrt'd)
            n1ps = mnp.tile([20, 4, 512], F32, name=f"n1ps_{d}", tag="n1ps")
            for ty in range(4):
                for c in range(2):
                    nc.tensor.matmul(
                        n1ps[:, ty, :], w2T[:, c, 4 * di + ty, :],
                        csq[:, c, :, :].rearrange("k s t -> k (s t)"),
                        start=(c == 0), stop=(c == 1))
            n1 = mp.tile([20, 4, 8, 64], F32, name=f"n1_{d}", tag="n1")
            nc.scalar.activation(n1.rearrange("l a b c -> l (a b c)"),
                                 n1ps.rearrange("l a b -> l (a b)"), AF.Sqrt)
            # clamped reciprocals of all norms (same layout)
            rn1 = mp.tile([20, 4, 8, 64], F32, name=f"rn1_{d}", tag="rn1")
            nc.vector.tensor_scalar_max(
                rn1.rearrange("l a b c -> l (a b c)"),
                n1.rearrange("l a b c -> l (a b c)"), EPS)
            nc.vector.reciprocal(rn1.rearrange("l a b c -> l (a b c)"),
                                 rn1.rearrange("l a b c -> l (a b c)"))

            # t-major con (64t, 8s, 256h) f32r
            ctm = mp.tile([64, 8, 256], F32R, name=f"ctm_{d}", tag="ctm")
            for s in range(8):
                tp = mnp.tile([64, 2, 128], F32R, name=f"ct_{d}_{s}",
                              tag="ctmp")
                for c in range(2):
                    nc.tensor.transpose(tp[:, c, :], cT[:, c, s, :],
                                        idr[:])
                nc.vector.tensor_copy(ctm[:, s, :],
                                      tp.rearrange("t c k -> t (c k)"))

            # per-t vector norms (attention), t-major: rvn (64, 8) recip-clamp
            ctmsq = mp.tile([64, 8, 256], F32, name=f"cts_{d}", tag="ctmsq")
            nc.scalar.activation(ctmsq.rearrange("t s h -> t (s h)"),
                                 ctm.rearrange("t s h -> t (s h)"), AF.Square)
            rvn = mp.tile([64, 8], F32, name=f"rvn_{d}", tag="rvn")
            nc.vector.tensor_reduce(out=rvn[:], in_=ctmsq[:], axis=AX_X,
                                    op=ALU.add)
            nc.scalar.activation(rvn[:], rvn[:], AF.Sqrt)
            nc.vector.tensor_scalar_max(rvn[:], rvn[:], EPS)
            nc.vector.reciprocal(rvn[:], rvn[:])

            # ---- FULL
            ancv = mn.tile([128, 2, 8], F32, name=f"ancv_{d}", tag="ancv")
            nc.vector.tensor_copy(ancv[:], cT[:, :, :, anchor_t])
            for b in range(BL):
                for ri, (s_me, s_an) in enumerate(((b, BL + b), (BL + b, b))):
                    anc = mn.tile([128, 2, 20], F32R, name=f"an_{d}_{b}_{ri}",
                                  tag="anc")
                    for c in range(2):
                        nc.vector.tensor_scalar_mul(
                            anc[:, c, :], w2T[:, c, 4 * di + FULL, :],
                            ancv[:, c, s_an:s_an + 1])
                    nps = mnp.tile([20, 64], F32, name=f"nf_{d}_{b}_{ri}",
                                   tag="nf")
                    for c in range(2):
                        nc.tensor.matmul(nps[:], anc[:, c, :],
                                         cT[:, c, s_me, :],
                                         start=(c == 0), stop=(c == 1))
                    den = mn.tile([20, 64], F32, name=f"de_{d}_{b}_{ri}",
                                  tag="den")
                    nc.vector.tensor_scalar(
                        out=den[:], in0=n1[:, FULL, s_me, :],
                        scalar1=n1[:, FULL, s_an, anchor_t:anchor_t + 1],
                        scalar2=EPS, op0=ALU.mult, op1=ALU.max)
                    nc.vector.reciprocal(den[:], den[:])
                    nc.vector.tensor_tensor(out=mcol(mt, 0, ri, b), in0=nps[:],
                                            in1=den[:], op=ALU.mult)

        # ---- MAX (pair): scaled builds
        with tc.tile_pool(name=f"mx_{d}", bufs=1) as mxp, \
             tc.tile_pool(name=f"mxps_{d}", bufs=1, space="PSUM") as mxps:
            # DRAM staging of clamped-recip norms (MAX type) per role
            nbc = {}
            for ri, s0 in ((0, 0), (1, BL)):
                dscr = dramp.tile([20, 256], F32, name=f"nd_{d}_{ri}")
                nc.sync.dma_start(
                    dscr[:], rn1[:, MAX, s0:s0 + BL, :].rearrange(
                        "l s t -> l (s t)"))
                nbc[ri] = dscr
            for bp in range(2):
                scp = mxp.tile([128, 20, 2, 2, 64], BF16,
                               name=f"scp_{d}_{bp}", tag="scp")
                sch = mxp.tile([128, 20, 2, 2, 64], BF16,
                               name=f"sch_{d}_{bp}", tag="sch")
                for l in range(L):
                    for c in range(2):
                        e1 = nc.vector if (l + c) % 2 == 0 else nc.gpsimd
                        e2 = nc.gpsimd if (l + c) % 2 == 0 else nc.vector
                        e1.tensor_scalar_mul(
                            scp[:, l, c, :, :],
                            cbf[:, c, 2 * bp:2 * bp + 2, :],
                            w2Tf[:, c, 4 * di + MAX, l:l + 1])
                        e2.tensor_scalar_mul(
                            sch[:, l, c, :, :],
                            cbf[:, c, BL + 2 * bp:BL + 2 * bp + 2, :],
                            w2Tf[:, c, 4 * di + MAX, l:l + 1])
                mxs_all = {}
                for side in range(2):
                    for b in (2 * bp, 2 * bp + 1):
                        mxs_all[(side, b)] = mxp.tile(
                            [64, 2, 10], F32, name=f"mxs_{d}_{b}_{side}",
                            tag=f"mxs_{side}_{b % 2}")
                for side in range(2):  # 0: max over j (p rows); 1: over i
                    sc_l = scp if side == 0 else sch
                    rs0 = BL if side == 0 else 0
                    for hf in range(2):
                        pps = mxps.tile([128, 10, 128], F32,
                                        name=f"pp_{d}_{bp}_{side}_{hf}",
                                        tag="pps", bufs=2)
                        for u in range(10):
                            l = 10 * hf + u
                            for c in range(2):
                                nc.tensor.matmul(
                                    pps[:, u, :],
                                    sc_l[:, l, c, :, :].rearrange(
                                        "k e t -> k (e t)"),
                                    cbf[:, c, rs0 + 2 * bp:rs0 + 2 * bp + 2,
                                        :].rearrange("k e t -> k (e t)"),
                                    start=(c == 0), stop=(c == 1))
                        for b in (2 * bp, 2 * bp + 1):
                            beta = b % 2
                            src_d = nbc[1] if side == 0 else nbc[0]
                            nbcb = mxp.tile([64, 10, 64], F32,
                                            name=f"nb_{d}_{b}_{side}_{hf}",
                                            tag="nbcb", bufs=2)
                            nc.sync.dma_start(
                                nbcb[:],
                                bass.AP(tensor=src_d.tensor,
                                        offset=src_d.offset
                                        + hf * 10 * 256 + b * 64,
                                        ap=[[0, 64], [256, 10], [1, 64]]))
                            pn = mxp.tile([64, 10, 64], BF16,
                                          name=f"pn_{d}_{b}_{side}_{hf}",
                                          tag="pn", bufs=2)
                            nc.vector.tensor_tensor(
                                out=pn[:],
                                in0=pps[64 * beta:64 * beta + 64, :,
                                        64 * beta:64 * beta + 64],
                                in1=nbcb[:], op=ALU.mult)
                            nc.vector.tensor_reduce(
                                out=mxs_all[(side, b)][:, hf, :], in_=pn[:],
                                axis=AX_X, op=ALU.max)
                for side in range(2):
                    for b in (2 * bp, 2 * bp + 1):
                        yt = mxps.tile([20, 64], F32,
                                       name=f"yt_{d}_{b}_{side}", tag="yt",
                                       bufs=2)
                        nc.tensor.transpose(
                            yt[:],
                            mxs_all[(side, b)].rearrange(
                                "t hf u -> t (hf u)"),
                            idf[0:64, 0:64])
                        ri_me = 0 if side == 0 else 1
                        s_me = b if side == 0 else BL + b
                        nc.vector.tensor_tensor(
                            out=mcol(mt, 32, ri_me, b), in0=yt[:],
                            in1=rn1[:, MAX, s_me, :], op=ALU.mult)

        # ---- AM + AX per (b)
        with tc.tile_pool(name=f"am_{d}", bufs=2) as amp, \
             tc.tile_pool(name=f"amps_{d}", bufs=2, space="PSUM") as amps:
            for b in range(BL):
                _am_ax_block(nc, tc, d, di, b, cT, cbf, csq, w2T, n1, rn1,
                             rvn, ctm, mvT[di], idf, amp, amps, dramp, mcol,
                             anchor_t)


def _am_ax_block(nc, tc, d, di, b, cT, cbf, csq, w2T, n1, rn1, rvn, ctm,
                 mt, idf, amp, amps, dramp, mcol, anchor_t):
    AM_SLOT, AX_SLOT = 64, 96
    AMTY, AXTY = 2, 3
    sp, sh = b, 4 + b

    # raw attention + transpose
    att_ps = amps.tile([64, 64], F32, name=f"at_{d}_{b}", tag="t64",
                       bufs=3)
    for c in range(2):
        nc.tensor.matmul(att_ps[:], cT[:, c, sp, :], cT[:, c, sh, :],
                         start=(c == 0), stop=(c == 1))
    # normalize: att_norm = rvn_p[i] * att * rvn_h[j]
    a1 = amp.tile([64, 64], F32, name=f"a1_{d}_{b}", tag="a1")
    nc.vector.tensor_scalar_mul(a1[:], att_ps[:], rvn[:, sp:sp + 1])
    a1t_ps = amps.tile([64, 64], F32, name=f"a1t_{d}_{b}", tag="t64", bufs=3)
    nc.tensor.transpose(a1t_ps[:], a1[:], idf[0:64, 0:64])
    attTn = amp.tile([64, 64], F32, name=f"aTn_{d}_{b}", tag="attTn")
    nc.vector.tensor_scalar_mul(attTn[:], a1t_ps[:], rvn[:, sh:sh + 1])
    attn_ps = amps.tile([64, 64], F32, name=f"an2_{d}_{b}", tag="t64", bufs=3)
    nc.tensor.transpose(attn_ps[:], attTn[:], idf[0:64, 0:64])
    attn = amp.tile([64, 64], F32, name=f"an_{d}_{b}", tag="attn")
    nc.scalar.activation(attn[:], attn_ps[:], AF.Copy)

    # row sums + clamped recips
    rs_h = amp.tile([64, 1], F32, name=f"rh_{d}_{b}", tag="rsh")
    nc.vector.tensor_reduce(out=rs_h[:], in_=attn[:], axis=AX_X, op=ALU.add)
    nc.vector.tensor_scalar_max(rs_h[:], rs_h[:], EPS)
    nc.vector.reciprocal(rs_h[:], rs_h[:])
    rs_p = amp.tile([64, 1], F32, name=f"rp_{d}_{b}", tag="rsp")
    nc.vector.tensor_reduce(out=rs_p[:], in_=attTn[:], axis=AX_X, op=ALU.add)
    nc.vector.tensor_scalar_max(rs_p[:], rs_p[:], EPS)
    nc.vector.reciprocal(rs_p[:], rs_p[:])

    # weighted mean rhs: AhT = T(attn * rs_h), BpT = T(attTn * rs_p)
    ah = amp.tile([64, 64], F32, name=f"ah_{d}_{b}", tag="ah")
    nc.vector.tensor_scalar_mul(ah[:], attn[:], rs_h[:, 0:1])
    ahT_ps = amps.tile([64, 64], F32, name=f"ahT_{d}_{b}", tag="t64", bufs=3)
    nc.tensor.transpose(ahT_ps[:], ah[:], idf[0:64, 0:64])
    ahT = amp.tile([64, 64], F32R, name=f"ahTs_{d}_{b}", tag="ahTs")
    nc.scalar.activation(ahT[:], ahT_ps[:], AF.Copy)
    bp = amp.tile([64, 64], F32, name=f"bp_{d}_{b}", tag="bp")
    nc.vector.tensor_scalar_mul(bp[:], attTn[:], rs_p[:, 0:1])
    bpT_ps = amps.tile([64, 64], F32, name=f"bpT_{d}_{b}", tag="t64", bufs=3)
    nc.tensor.transpose(bpT_ps[:], bp[:], idf[0:64, 0:64])
    bpT = amp.tile([64, 64], F32R, name=f"bpTs_{d}_{b}", tag="bpTs")
    nc.scalar.activation(bpT[:], bpT_ps[:], AF.Copy)

    # am vectors (hd-major): am_hT[hc][h,i], am_pT[hc][h,j]
    for role, (rhs, s_ctm, s_me, coln) in enumerate(
            ((ahT, sh, sp, 0), (bpT, sp, sh, 1))):
        amv_ps = amps.tile([128, 2, 64], F32, name=f"av_{d}_{b}_{role}",
                           tag="amv", bufs=1)
        for c in range(2):
            nc.tensor.matmul(amv_ps[:, c, :],
                             ctm[:, s_ctm, 128 * c:128 * (c + 1)], rhs[:],
                             start=True, stop=True)
        amv = amp.tile([128, 2, 64], F32R, name=f"am_{d}_{b}_{role}",
                       tag="amv_sb")
        nc.scalar.activation(amv.rearrange("k c t -> k (c t)"),
                             amv_ps.rearrange("k c t -> k (c t)"), AF.Copy)
        # num = W2 @ (v * am)
        prod = amp.tile([128, 2, 64], F32R, name=f"pr_{d}_{b}_{role}",
                        tag="prod")
        for c in range(2):
            nc.vector.tensor_tensor(out=prod[:, c, :], in0=cT[:, c, s_me, :],
                                    in1=amv[:, c, :], op=ALU.mult)
        nump = amps.tile([20, 64], F32, name=f"nu_{d}_{b}_{role}", tag="s20", bufs=2)
        for c in range(2):
            nc.tensor.matmul(nump[:], w2T[:, c, 4 * di + AMTY, :],
                             prod[:, c, :], start=(c == 0), stop=(c == 1))
        # n2 = sqrt(W2 @ am^2)
        amsq = amp.tile([128, 2, 64], F32R, name=f"as_{d}_{b}_{role}",
                        tag="amsq")
        nc.scalar.activation(amsq.rearrange("k c t -> k (c t)"),
                             amv.rearrange("k c t -> k (c t)"), AF.Square)
        n2p = amps.tile([20, 64], F32, name=f"n2_{d}_{b}_{role}", tag="s20", bufs=2)
        for c in range(2):
            nc.tensor.matmul(n2p[:], w2T[:, c, 4 * di + AMTY, :],
                             amsq[:, c, :], start=(c == 0), stop=(c == 1))
        n2s = amp.tile([20, 64], F32, name=f"ns_{d}_{b}_{role}", tag="n2s")
        nc.scalar.activation(n2s[:], n2p[:], AF.Sqrt)
        den = amp.tile([20, 64], F32, name=f"dn_{d}_{b}_{role}", tag="amden")
        nc.vector.tensor_tensor(out=den[:], in0=n1[:, AMTY, s_me, :],
                                in1=n2s[:], op=ALU.mult)
        nc.vector.tensor_scalar_max(den[:], den[:], EPS)
        nc.vector.reciprocal(den[:], den[:])
        nc.vector.tensor_tensor(out=mcol(mt, AM_SLOT, role, b), in0=nump[:],
                                in1=den[:], op=ALU.mult)

    # ---- AX: att_h.max / att_p.max via DRAM-broadcast of attn
    atb = amp.tile([64, 64], BF16, name=f"ab_{d}_{b}", tag="atb")
    nc.vector.tensor_copy(atb[:], attn[:])
    atbT = amp.tile([64, 64], BF16, name=f"abT_{d}_{b}", tag="atbT")
    nc.vector.tensor_copy(atbT[:], attTn[:])
    dsc = dramp.tile([64, 64], BF16, name=f"dx_{d}_{b}")
    nc.sync.dma_start(dsc[:], atb[:])
    dscT = dramp.tile([64, 64], BF16, name=f"dxT_{d}_{b}")
    nc.sync.dma_start(dscT[:], atbT[:])
    bch = amp.tile([128, 64, 64], BF16, name=f"bc_{d}_{b}", tag="bch", bufs=1)
    nc.sync.dma_start(bch[:], bass.AP(tensor=dsc.tensor, offset=dsc.offset,
                                      ap=[[0, 128], [64, 64], [1, 64]]))
    bcp = amp.tile([128, 64, 64], BF16, name=f"bcT_{d}_{b}", tag="bcp", bufs=1)
    nc.sync.dma_start(bcp[:], bass.AP(tensor=dscT.tensor, offset=dscT.offset,
                                      ap=[[0, 128], [64, 64], [1, 64]]))

    for role in range(2):
        s_v = sh if role == 0 else sp      # the "other" sequence vectors
        s_me = sp if role == 0 else sh
        bc = bch if role == 0 else bcp
        axm = amp.tile([128, 2, 64], F32R, name=f"axm_{d}_{b}_{role}",
                       tag="axm")
        for c in range(2):
            eng = nc.vector if (b + role + c) % 2 == 0 else nc.gpsimd
            prod = amp.tile([128, 64, 64], BF16,
                            name=f"xp_{d}_{b}_{role}_{c}", tag="xprod", bufs=1)
            vb = cbf[:, c, s_v, :]
            eng.tensor_tensor(
                out=prod[:],
                in0=bass.AP(tensor=vb.tensor, offset=vb.offset,
                            ap=[vb.ap[0], [0, 64], vb.ap[1]]),
                in1=bc[:], op=ALU.mult)
            nc.vector.tensor_reduce(out=axm[:, c, :], in_=prod[:],
                                    axis=AX_X, op=ALU.max)
        # cos(vp, axm) under w_ax
        prodx = amp.tile([128, 2, 64], F32R, name=f"px_{d}_{b}_{role}",
                         tag="prodx")
        for c in range(2):
            nc.vector.tensor_tensor(out=prodx[:, c, :], in0=cT[:, c, s_me, :],
                                    in1=axm[:, c, :], op=ALU.mult)
        nux = amps.tile([20, 64], F32, name=f"nx_{d}_{b}_{role}", tag="s20", bufs=2)
        for c in range(2):
            nc.tensor.matmul(nux[:], w2T[:, c, 4 * di + AXTY, :],
                             prodx[:, c, :], start=(c == 0), stop=(c == 1))
        axsq = amp.tile([128, 2, 64], F32R, name=f"xs_{d}_{b}_{role}",
                        tag="axsq")
        nc.scalar.activation(axsq.rearrange("k c t -> k (c t)"),
                             axm.rearrange("k c t -> k (c t)"), AF.Square)
        n2x = amps.tile([20, 64], F32, name=f"n2x_{d}_{b}_{role}", tag="s20", bufs=2)
        for c in range(2):
            nc.tensor.matmul(n2x[:], w2T[:, c, 4 * di + AXTY, :],
                             axsq[:, c, :], start=(c == 0), stop=(c == 1))
        n2xs = amp.tile([20, 64], F32, name=f"nxs_{d}_{b}_{role}", tag="n2xs")
        nc.scalar.activation(n2xs[:], n2x[:], AF.Sqrt)
        den = amp.tile([20, 64], F32, name=f"dx2_{d}_{b}_{role}", tag="axden")
        nc.vector.tensor_tensor(out=den[:], in0=n1[:, AXTY, s_me, :],
                                in1=n2xs[:], op=ALU.mult)
        nc.vector.tensor_scalar_max(den[:], den[:], EPS)
        nc.vector.reciprocal(den[:], den[:])
        nc.vector.tensor_tensor(out=mcol(mt, AX_SLOT, role, b), in0=nux[:],
                                in1=den[:], op=ALU.mult)


# ---------------------------------------------------------------- entry

def _get_nc(debug=False):
    key = ('dbg' if debug else 'rel')
    if key not in _CACHE:
        _CACHE[key] = build_nc(debug)
    return _CACHE[key]


def kernel(**inputs):
    nc = _get_nc(False)
    w = _prep_weights(inputs)
    in_maps = []
    for core in range(NCORES):
        m = dict(w)
        m['tokp'] = _prep_tokens(inputs['q1_inputs'], inputs['q2_inputs'],
                                 core)
        in_maps.append(m)
    res = run_bass_kernel_spmd(nc, in_maps, core_ids=list(range(NCORES)))
    out = np.concatenate([res.results[c]['y'] for c in range(NCORES)], axis=0)
    return out.astype(np.float32)


def run_debug(inputs):
    nc = _get_nc(True)
    w = _prep_weights(inputs)
    in_maps = []
    for core in range(NCORES):
        m = dict(w)
        m['tokp'] = _prep_tokens(inputs['q1_inputs'], inputs['q2_inputs'],
                                 core)
        in_maps.append(m)
    res = run_bass_kernel_spmd(nc, in_maps, core_ids=list(range(NCORES)))
    return res


# revision 16
# speedup vs baseline: 2.0045x; 2.0045x over previous
"""BiMPM Trainium2 Bass kernel — pure data parallel over batch (B=32 -> 4/core).

Per-core layouts (B_l=4, stack S=8 rows per step = [p:b0..3, h:b0..3]):
- token/row order: r = t*8 + s, s = seq*4 + b (seq0 = q1 = "p", seq1 = q2 = "h")
- xg projections: (128 = 16t x 8s, m=4, 1024) bf16 per dir
- gates psum: fw rows [0:8], bw rows [32:40] (col-tiled bf16 matmuls)
- scan state c/h: (64p, 256) f32, rows [0:8] fw / [32:40] bw
- conT (ctx outputs, hd-major): (128 = hd%128, 2c, 8s, 64t) f32r per dir
- mvT (match features): 2 tiles (128, 512 = 8t*64... cols r = 8t+s) f32r,
  feature rows at 32-aligned slots [full@0, max@32, am@64, ax@96, ones@116]
"""
import numpy as np
from contextlib import ExitStack

import concourse.bass as bass
import concourse.tile as tile
from concourse import bacc, mybir
from concourse.bass_utils import run_bass_kernel_spmd
from concourse.masks import make_identity

F32 = mybir.dt.float32
F32R = mybir.dt.float32r
BF16 = mybir.dt.bfloat16
I32 = mybir.dt.int32
AF = mybir.ActivationFunctionType
ALU = mybir.AluOpType
AX_X = mybir.AxisListType.X

B, T, V, D, H, L, NL = 32, 64, 50000, 300, 256, 20, 2
NCORES = 8
BL = B // NCORES
S = 2 * BL
EPS = 1e-8

_CACHE = {}
PHASES = 'full'  # 'ctx' | 'match' | 'full' (for TimelineSim bisection)


# ---------------------------------------------------------------- host prep

def _gate_reorder(w):
    i, f, g, o = np.split(w, 4, axis=0)
    return np.concatenate([i, f, o, g], axis=0)


def _prep_weights(inp):
    w = {}
    f32 = np.float32

    def ctx_wT(dir_):
        # ws layout: [k%128, kc(3), gc(8), m(128)]; row 300 = bias, pad to 384
        wih = _gate_reorder(np.asarray(inp[f'ctx_wih_{dir_}'], f32))
        bias = _gate_reorder(
            np.asarray(inp[f'ctx_bih_{dir_}'] + inp[f'ctx_bhh_{dir_}'],
                       f32)[:, None]).T
        wt = np.concatenate([wih.T, bias, np.zeros((83, 1024), f32)], 0)
        return np.ascontiguousarray(
            wt.reshape(3, 128, 8, 128).transpose(1, 0, 2, 3), f32)

    def whhT(pfx, dir_):
        # ws layout: [k%128, kc, gc, m] = whh_reord[gc*128+m, kc*128+k]
        whh = _gate_reorder(np.asarray(inp[f'{pfx}_whh_{dir_}'], f32))
        return np.ascontiguousarray(
            whh.T.reshape(2, 128, 8, 128).transpose(1, 0, 2, 3), f32)

    w['wihT_f'], w['wihT_b'] = ctx_wT('f'), ctx_wT('b')
    w['whhT_f'], w['whhT_b'] = whhT('ctx', 'f'), whhT('ctx', 'b')
    w['awhhT_f'], w['awhhT_b'] = whhT('agg', 'f'), whhT('agg', 'b')

    def agg_wT(dir_):
        wih = _gate_reorder(np.asarray(inp[f'agg_wih_{dir_}'], f32))
        bias = _gate_reorder(
            np.asarray(inp[f'agg_bih_{dir_}'] + inp[f'agg_bhh_{dir_}'],
                       f32)[:, None]).T
        out = np.zeros((256, 1024), f32)
        for d in range(2):
            for ty in range(4):
                src = wih[:, d * 80 + ty * 20: d * 80 + ty * 20 + 20]
                out[d * 128 + 32 * ty: d * 128 + 32 * ty + 20] = src.T
        out[116] = bias[0]
        return np.ascontiguousarray(
            out.reshape(2, 128, 8, 128).transpose(1, 0, 2, 3), f32)

    w['aggwT_f'], w['aggwT_b'] = agg_wT('f'), agg_wT('b')

    w2 = np.asarray(inp['mp_w'], f32) ** 2
    w2t = np.zeros((128, 2, 8, 20), f32)
    for d in range(2):
        for ty in range(4):
            src = w2[2 * ty + d]
            for c in range(2):
                w2t[:, c, d * 4 + ty, :] = src[:, c * 128:(c + 1) * 128].T
    w['w2T'] = np.ascontiguousarray(w2t)

    fc1 = np.asarray(inp['fc1_w'], f32)
    w['fc1T'] = np.ascontiguousarray(
        fc1.T.reshape(8, 128, 512).transpose(1, 0, 2))
    w['fc1b'] = np.ascontiguousarray(
        np.broadcast_to(np.asarray(inp['fc1_b'], f32), (BL, 512)))
    fc2 = np.asarray(inp['fc2_w'], f32)
    w['fc2T'] = np.ascontiguousarray(
        fc2.T.reshape(4, 128, 2).transpose(1, 0, 2))
    w['fc2b'] = np.ascontiguousarray(
        np.broadcast_to(np.asarray(inp['fc2_b'], f32), (BL, 2)))
    w['word_emb'] = np.ascontiguousarray(np.asarray(inp['word_emb'], f32))
    return w


def _prep_tokens(q1, q2, core):
    q1c = np.asarray(q1[core * BL:(core + 1) * BL]).astype(np.int64)
    q2c = np.asarray(q2[core * BL:(core + 1) * BL]).astype(np.int64)
    tok = np.zeros((T * S,), np.int32)
    for seq, q in ((0, q1c), (1, q2c)):
        for b in range(BL):
            tok[np.arange(T) * S + seq * BL + b] = q[b]
    return np.ascontiguousarray(tok.reshape(4, 128))


# ---------------------------------------------------------------- build

def build_nc(debug=False):
    nc = bacc.Bacc("TRN2", target_bir_lowering=False, debug=False,
                   enable_asserts=True, num_devices=NCORES)
    dt = nc.dram_tensor
    dr = {}
    dr['tokp'] = dt("tokp", [4, 128], I32, kind="ExternalInput").ap()
    dr['word_emb'] = dt("word_emb", [V, D], F32, kind="ExternalInput").ap()
    for n, shp in [('wihT_f', [128, 3, 8, 128]), ('wihT_b', [128, 3, 8, 128]),
                   ('whhT_f', [128, 2, 8, 128]), ('whhT_b', [128, 2, 8, 128]),
                   ('awhhT_f', [128, 2, 8, 128]), ('awhhT_b', [128, 2, 8, 128]),
                   ('aggwT_f', [128, 2, 8, 128]), ('aggwT_b', [128, 2, 8, 128]),
                   ('w2T', [128, 2, 8, 20]), ('fc1T', [128, 8, 512]),
                   ('fc1b', [BL, 512]), ('fc2T', [128, 4, 2]),
                   ('fc2b', [BL, 2])]:
        dr[n] = dt(n, shp, F32, kind="ExternalInput").ap()
    y = dt("y", [BL, NL], F32, kind="ExternalOutput").ap()
    dbg = {}
    if debug:
        dbg['conT_f'] = dt("dbg_conT_f", [128, 2, 8, 64], F32,
                           kind="ExternalOutput").ap()
        dbg['conT_b'] = dt("dbg_conT_b", [128, 2, 8, 64], F32,
                           kind="ExternalOutput").ap()
        dbg['mvT0'] = dt("dbg_mvT0", [128, 512], F32,
                         kind="ExternalOutput").ap()
        dbg['mvT1'] = dt("dbg_mvT1", [128, 512], F32,
                         kind="ExternalOutput").ap()
        dbg['xT'] = dt("dbg_xT", [128, 2, 40], F32,
                       kind="ExternalOutput").ap()

    with tile.TileContext(nc) as tc, ExitStack() as ctx:
        _body(nc, tc, ctx, dr, y, dbg)
    nc.compile()
    return nc


def _body(nc, tc, ctx, dr, y, dbg):
    perm = ctx.enter_context(tc.tile_pool(name="perm", bufs=1))

    idf = perm.tile([128, 128], F32, name="idf")
    make_identity(nc, idf[:])
    idb = perm.tile([128, 128], BF16, name="idb")
    nc.vector.tensor_copy(idb[:], idf[:])
    selb = idb.rearrange("k (tl s) -> k tl s", s=8)
    idr4 = perm.tile([4, 4], F32R, name="idr4")
    nc.vector.tensor_copy(idr4[:], idf[0:4, 0:4])
    idr = perm.tile([128, 128], F32R, name="idr")
    nc.gpsimd.tensor_copy(idr[:], idf[:])

    def conv(src, dtype, name, engine=None, pool=None):
        t = (pool or perm).tile(list(src.shape), dtype, name=f"C_{name}")
        eng = engine or nc.vector
        if eng is nc.scalar:
            eng.activation(t[:], src[:], AF.Copy)
        else:
            eng.tensor_copy(t[:], src[:])
        return t

    wihT, whhTb, awhhTb, aggwT = {}, {}, {}, {}
    w2Tf = perm.tile([128, 2, 8, 20], F32, name="w2Tf")
    nc.sync.dma_start(w2Tf[:], dr['w2T'][:])
    w2T = conv(w2Tf, F32R, "w2T")
    with tc.tile_pool(name="loadp", bufs=1) as loadp:
        def load_f32(name, shp, tag):
            t = loadp.tile(shp, F32, name=f"L_{name}", tag=tag)
            nc.sync.dma_start(t[:], dr[name][:])
            return t

        for d in 'fb':
            wihT[d] = conv(load_f32(f'wihT_{d}', [128, 3, 8, 128], "raw12k"),
                           F32R, f"wihT_{d}", nc.scalar)
            whhTb[d] = conv(load_f32(f'whhT_{d}', [128, 2, 8, 128], "raw8k"),
                            BF16, f"whh_{d}")
            awhhTb[d] = conv(load_f32(f'awhhT_{d}', [128, 2, 8, 128],
                                      "raw8k"), BF16, f"awhh_{d}", nc.gpsimd)
            aggwT[d] = conv(load_f32(f'aggwT_{d}', [128, 2, 8, 128], "raw8k"),
                            BF16, f"aggw_{d}", nc.scalar)

    idx_sb = perm.tile([128, 4], I32, name="idx_sb")
    nc.sync.dma_start(idx_sb[:], dr['tokp'].rearrange("m p -> p m"))

    # ---------------- embedding gather + ctx input projection (ws form)
    # xgT[d]: (128 = g%128, 8 gc, 512 rows) bf16 ; row r = t*8 + s
    xgT = {'f': perm.tile([128, 8, 512], BF16, name="xgT_f"),
           'b': perm.tile([128, 8, 512], BF16, name="xgT_b")}
    with tc.tile_pool(name="embp", bufs=2) as embp, \
         tc.tile_pool(name="epsum", bufs=2, space="PSUM") as epsum:
        embT = []
        for m in range(4):
            emb = embp.tile([128, 304], F32, name=f"emb_{m}", tag="emb")
            nc.gpsimd.indirect_dma_start(
                out=emb[:, 0:300], out_offset=None, in_=dr['word_emb'][:],
                in_offset=bass.IndirectOffsetOnAxis(ap=idx_sb[:, m:m + 1],
                                                    axis=0))
            nc.vector.memset(emb[:, 300:301], 1.0)
            et = embp.tile([128, 3, 128], F32R, name=f"embT_{m}", tag=f"eT{m}")
            for c in range(3):
                kc = min(128, 301 - 128 * c)
                tp = epsum.tile([128, 128], F32, name=f"etp_{m}_{c}",
                                tag="etp")
                nc.tensor.transpose(tp[0:kc, :], emb[:, 128 * c:128 * c + kc],
                                    idf[:])
                nc.scalar.activation(et[0:kc, c, :], tp[0:kc, :], AF.Copy)
            embT.append(et)
        for di, d in enumerate('fb'):
            for gc in range(8):
                ps = epsum.tile([128, 512], F32, name=f"xps_{d}_{gc}",
                                tag="xps")
                for m in range(4):
                    for c in range(3):
                        kc = min(128, 301 - 128 * c)
                        nc.tensor.matmul(
                            ps[:, 128 * m:128 * (m + 1)],
                            wihT[d][0:kc, c, gc, :], embT[m][0:kc, c, :],
                            start=(c == 0), stop=(c == 2))
                if gc % 2 == 0:
                    nc.vector.tensor_copy(xgT[d][:, gc, :], ps[:])
                else:
                    nc.scalar.activation(xgT[d][:, gc, :], ps[:], AF.Copy)

    # ---------------- scan layer (shared ctx/agg), weights-stationary form
    # state h/c: (128 = hd%128, 2 kc, 8 s); gates psum: (128 = g%128, 8 gc, 8)
    # gate chunk order: [i0 i1 f0 f1 o0 o1 g0 g1]
    def scan_layer(xgd, whh_d, conT_out, hfin, lname):
        sp = ctx2.enter_context(tc.tile_pool(name=f"sp_{lname}", bufs=3))
        pp = ctx2.enter_context(tc.tile_pool(name=f"pp_{lname}", bufs=2,
                                             space="PSUM"))
        cp = ctx2.enter_context(tc.tile_pool(name=f"cp_{lname}", bufs=1))
        c_sb = {d: cp.tile([128, 2, 8], F32, name=f"c_{lname}_{d}")
                for d in 'fb'}
        h_prev = {d: None for d in 'fb'}
        for tau in range(T):
            ts_ = {'f': tau, 'b': T - 1 - tau}
            for di, d in enumerate('fb'):
                t = ts_[d]
                xgs = xgd[d][:, :, 8 * t:8 * t + 8]        # (128, 8gc, 8)
                if h_prev[d] is None:
                    gsb = xgs
                else:
                    gps = pp.tile([128, 8, 8], F32, name=f"g_{lname}_{d}_{tau}",
                                  tag=f"gps_{d}")
                    for gc in range(8):
                        for kc in range(2):
                            nc.tensor.matmul(
                                gps[:, gc, :], whh_d[d][:, kc, gc, :],
                                h_prev[d][:, kc, :],
                                start=(kc == 0), stop=(kc == 1))
                    gsb = sp.tile([128, 8, 8], F32, name=f"gs_{lname}_{d}_{tau}",
                                  tag=f"gsb_{d}")
                    nc.vector.tensor_tensor(
                        out=gsb,
                        in0=gps,
                        in1=xgs, op=ALU.add)
                sig = sp.tile([128, 6, 8], F32, name=f"si_{lname}_{d}_{tau}",
                              tag=f"sig_{d}")
                nc.scalar.activation(sig,
                                     gsb[:, 0:6, :], AF.Sigmoid)
                tg = sp.tile([128, 2, 8], F32, name=f"tg_{lname}_{d}_{tau}",
                             tag=f"tg_{d}")
                nc.scalar.activation(tg,
                                     gsb[:, 6:8, :], AF.Tanh)
                cs = c_sb[d]
                if h_prev[d] is None:
                    nc.vector.tensor_tensor(
                        out=cs,
                        in0=sig[:, 0:2, :],
                        in1=tg, op=ALU.mult)
                else:
                    t1 = sp.tile([128, 2, 8], F32, name=f"t1_{lname}_{d}_{tau}",
                                 tag=f"t1_{d}")
                    nc.vector.tensor_tensor(
                        out=t1,
                        in0=sig[:, 2:4, :],
                        in1=cs, op=ALU.mult)
                    t2 = sp.tile([128, 2, 8], F32, name=f"t2_{lname}_{d}_{tau}",
                                 tag=f"t2_{d}")
                    nc.vector.tensor_tensor(
                        out=t2,
                        in0=sig[:, 0:2, :],
                        in1=tg, op=ALU.mult)
                    nc.vector.tensor_tensor(
                        out=cs,
                        in0=t1,
                        in1=t2, op=ALU.add)
                th = sp.tile([128, 2, 8], F32, name=f"th_{lname}_{d}_{tau}",
                             tag=f"th_{d}")
                nc.scalar.activation(th,
                                     cs,
                                     AF.Tanh)
                h_bf = sp.tile([128, 2, 8], BF16, name=f"h_{lname}_{d}_{tau}",
                               tag=f"h_{d}")
                nc.vector.tensor_tensor(
                    out=h_bf,
                    in0=sig[:, 4:6, :],
                    in1=th, op=ALU.mult)
                h_prev[d] = h_bf
                if conT_out is not None:
                    nc.gpsimd.tensor_tensor(
                        out=conT_out[d][:, :, :, t].rearrange(
                            "k a b -> k (a b)"),
                        in0=sig[:, 4:6, :],
                        in1=th, op=ALU.mult)
                if hfin is not None and tau == T - 1:
                    nc.gpsimd.tensor_tensor(
                        out=hfin[d],
                        in0=sig[:, 4:6, :],
                        in1=th, op=ALU.mult)

    conT = {'f': perm.tile([128, 2, 8, 64], F32R, name="conT_f"),
            'b': perm.tile([128, 2, 8, 64], F32R, name="conT_b")}
    with ExitStack() as ctx2:
        scan_layer(xgT, whhTb, conT, None, "ctx")

    if PHASES == 'ctx':
        y_sb0 = perm.tile([BL, NL], F32, name="y_sb0")
        nc.vector.tensor_copy(y_sb0[:], conT['f'][0:BL, 0, 0, 0:NL])
        nc.sync.dma_start(y[:], y_sb0[:])
        return

    # ---------------- matching
    mvT = [perm.tile([128, 512], F32R, name="mvT0"),
           perm.tile([128, 512], F32R, name="mvT1")]
    # f32r memset unsupported; fill via ACT copy with scale=0 (+bias)
    fill_src = bass.AP(tensor=idf.tensor, offset=idf.offset,
                       ap=[idf.ap[0], [0, 512]])
    nc.scalar.activation(mvT[0][:], fill_src, AF.Copy, bias=0.0, scale=0.0)
    nc.scalar.activation(mvT[1][:], fill_src, AF.Copy, bias=0.0, scale=0.0)
    nc.scalar.activation(mvT[0][96:128, :],
                         bass.AP(tensor=idf.tensor, offset=idf.offset,
                                 ap=[[idf.ap[0][0], 32], [0, 512]]),
                         AF.Copy, bias=1.0, scale=0.0)
    _matching(nc, tc, ctx, conT, w2T, w2Tf, mvT, idf, idr)

    if PHASES == 'match':
        y_sb0 = perm.tile([BL, NL], F32, name="y_sb0")
        nc.vector.tensor_copy(y_sb0[:], mvT[0][0:BL, 0:NL])
        nc.sync.dma_start(y[:], y_sb0[:])
        return

    # ---------------- agg projection (ws form)
    xgaT = {'f': perm.tile([128, 8, 512], BF16, name="xgaT_f"),
            'b': perm.tile([128, 8, 512], BF16, name="xgaT_b")}
    with tc.tile_pool(name="aggps", bufs=2, space="PSUM") as ap_ps:
        mvbf = [perm.tile([128, 512], BF16, name=f"mvbf{i}") for i in (0, 1)]
        nc.vector.tensor_copy(mvbf[0][:], mvT[0][:])
        nc.vector.tensor_copy(mvbf[1][:], mvT[1][:])
        for di, d in enumerate('fb'):
            for gc in range(8):
                ps = ap_ps.tile([128, 512], F32, name=f"ap_{d}_{gc}",
                                tag="aps")
                for kc in range(2):
                    nc.tensor.matmul(ps[:], aggwT[d][:, kc, gc, :],
                                     mvbf[kc][:],
                                     start=(kc == 0), stop=(kc == 1))
                if gc % 2 == 0:
                    nc.vector.tensor_copy(xgaT[d][:, gc, :], ps[:])
                else:
                    nc.scalar.activation(xgaT[d][:, gc, :], ps[:], AF.Copy)

    # ---------------- agg scans + fc
    hfin = {d: perm.tile([128, 2, 8], F32R, name=f"hfin_{d}") for d in 'fb'}
    with ExitStack() as ctx2:
        scan_layer(xgaT, awhhTb, None, hfin, "agg")

    fcp = ctx.enter_context(tc.tile_pool(name="fcp", bufs=1))
    fps = ctx.enter_context(tc.tile_pool(name="fcps", bufs=1, space="PSUM"))
    fc1T = fcp.tile([128, 8, 512], F32R, name="fc1T")
    fc2T = fcp.tile([128, 4, 2], F32R, name="fc2T")
    fc1b = fcp.tile([BL, 512], F32R, name="fc1b")
    fc2b = fcp.tile([BL, 2], F32R, name="fc2b")
    with tc.tile_pool(name="fcload", bufs=1) as fcl:
        for nm, tgt in (('fc1T', fc1T), ('fc2T', fc2T), ('fc1b', fc1b),
                        ('fc2b', fc2b)):
            rawf = fcl.tile(list(tgt.shape), F32, name=f"rf_{nm}", tag="rawfc")
            nc.sync.dma_start(rawf[:], dr[nm][:])
            nc.vector.tensor_copy(tgt[:], rawf[:])
    # x k-chunks: [hpf c0, hpf c1, hpb c0, hpb c1, hhf c0, hhf c1, hhb c0, hhb c1]
    ksl = []
    for role0 in (0, 4):
        for d in 'fb':
            for c in range(2):
                ksl.append(hfin[d][:, c, role0:role0 + BL])
    x1 = fps.tile([BL, 512], F32, name="x1")
    for kc in range(8):
        nc.tensor.matmul(x1[:], ksl[kc], fc1T[:, kc, :],
                         start=(kc == 0), stop=False)
    nc.tensor.matmul(x1[:], idr4[:], fc1b[:], start=False, stop=True)
    xt1 = fcp.tile([BL, 512], F32, name="xt1")
    nc.scalar.activation(xt1[:], x1[:], AF.Tanh)
    xt1ps = fps.tile([128, 4, BL], F32, name="xt1ps")
    for c in range(4):
        nc.tensor.transpose(xt1ps[:, c, :], xt1[:, 128 * c:128 * (c + 1)],
                            idf[0:BL, 0:BL])
    xt1T = fcp.tile([128, 4, BL], F32R, name="xt1T")
    nc.vector.tensor_copy(xt1T[:], xt1ps[:])
    yps = fps.tile([BL, NL], F32, name="yps")
    for c in range(4):
        nc.tensor.matmul(yps[:], xt1T[:, c, :], fc2T[:, c, :],
                         start=(c == 0), stop=False)
    nc.tensor.matmul(yps[:], idr4[:], fc2b[:], start=False, stop=True)
    y_sb = fcp.tile([BL, NL], F32, name="y_sb")
    nc.vector.tensor_copy(y_sb[:], yps[:])
    nc.sync.dma_start(y[:], y_sb[:])

    if dbg:
        for d in 'fb':
            cf = fcp.tile([128, 2, 8, 64], F32, name=f"dbgc_{d}")
            nc.scalar.activation(cf[:], conT[d][:], AF.Copy)
            nc.sync.dma_start(dbg[f'conT_{d}'][:], cf[:])
        for i in range(2):
            mf = fcp.tile([128, 512], F32, name=f"dbgm_{i}")
            nc.scalar.activation(mf[:], mvT[i][:], AF.Copy)
            nc.sync.dma_start(dbg[f'mvT{i}'][:], mf[:])
        xtd = fcp.tile([128, 2, 40], F32, name="xtd")
        nc.vector.memset(xtd[:], 0.0)
        nc.vector.tensor_copy(xtd[:, :, 0:8], hfin['f'][:])
        nc.vector.tensor_copy(xtd[:, :, 32:40], hfin['b'][:])
        nc.sync.dma_start(dbg['xT'][:], xtd[:])


# ---------------------------------------------------------------- matching

def _matching(nc, tc, ctx, conT, w2T, w2Tf, mvT, idf, idr):
    FULL, MAX, AM, AX = 0, 1, 2, 3
    mp = ctx.enter_context(tc.tile_pool(name="mp", bufs=1))
    dramp = ctx.enter_context(tc.tile_pool(name="mdram", bufs=1,
                                           space="DRAM"))

    def mcol(mt, slot, ri, b):
        # (20, 64) view of mvT rows [slot:slot+20], cols 8t + ri*4 + b
        return mt[slot:slot + 20, :].rearrange("l (t s) -> l t s",
                                               s=8)[:, :, ri * BL + b]

    for di, d in enumerate('fb'):
        cT = conT[d]
        anchor_t = (T - 1) if d == 'f' else 0
        mt = mvT[di]

        with tc.tile_pool(name=f"mn_{d}", bufs=1) as mn, \
             tc.tile_pool(name=f"mnp_{d}", bufs=1, space="PSUM") as mnp:
            # squares, bf16 copy
            csq = mp.tile([128, 2, 8, 64], F32R, name=f"csq_{d}", tag="csq")
            nc.scalar.activation(csq[:], cT[:], AF.Square)
            cbf = mp.tile([128, 2, 8, 64], BF16, name=f"cbf_{d}", tag="cbf")
            nc.vector.tensor_copy(cbf[:], cT[:])

            # norms n1[l, ty, s, t] (f32, sqrt'd)
            n1ps = mnp.tile([20, 4, 512], F32, name=f"n1ps_{d}", tag="n1ps")
            for ty in range(4):
                for c in range(2):
                    nc.tensor.matmul(
                        n1ps[:, ty, :], w2T[:, c, 4 * di + ty, :],
                        csq[:, c, :, :].rearrange("k s t -> k (s t)"),
                        start=(c == 0), stop=(c == 1))
            n1 = mp.tile([20, 4, 8, 64], F32, name=f"n1_{d}", tag="n1")
            nc.scalar.activation(n1.rearrange("l a b c -> l (a b c)"),
                                 n1ps.rearrange("l a b -> l (a b)"), AF.Sqrt)
            # clamped reciprocals of all norms (same layout)
            rn1 = mp.tile([20, 4, 8, 64], F32, name=f"rn1_{d}", tag="rn1")
            nc.vector.tensor_scalar_max(
                rn1.rearrange("l a b c -> l (a b c)"),
                n1.rearrange("l a b c -> l (a b c)"), EPS)
            nc.vector.reciprocal(rn1.rearrange("l a b c -> l (a b c)"),
                                 rn1.rearrange("l a b c -> l (a b c)"))

            # t-major con (64t, 8s, 256h) f32r
            ctm = mp.tile([64, 8, 256], F32R, name=f"ctm_{d}", tag="ctm")
            for s in range(8):
                tp = mnp.tile([64, 2, 128], F32R, name=f"ct_{d}_{s}",
                              tag="ctmp")
                for c in range(2):
                    nc.tensor.transpose(tp[:, c, :], cT[:, c, s, :],
                                        idr[:])
                nc.vector.tensor_copy(ctm[:, s, :],
                                      tp.rearrange("t c k -> t (c k)"))

            # per-t vector norms (attention), t-major: rvn (64, 8) recip-clamp
            ctmsq = mp.tile([64, 8, 256], F32, name=f"cts_{d}", tag="ctmsq")
            nc.scalar.activation(ctmsq.rearrange("t s h -> t (s h)"),
                                 ctm.rearrange("t s h -> t (s h)"), AF.Square)
            rvn = mp.tile([64, 8], F32, name=f"rvn_{d}", tag="rvn")
            nc.vector.tensor_reduce(out=rvn[:], in_=ctmsq[:], axis=AX_X,
                                    op=ALU.add)
            nc.scalar.activation(rvn[:], rvn[:], AF.Sqrt)
            nc.vector.tensor_scalar_max(rvn[:], rvn[:], EPS)
            nc.vector.reciprocal(rvn[:], rvn[:])

            # ---- FULL
            ancv = mn.tile([128, 2, 8], F32, name=f"ancv_{d}", tag="ancv")
            nc.vector.tensor_copy(ancv[:], cT[:, :, :, anchor_t])
            for b in range(BL):
                for ri, (s_me, s_an) in enumerate(((b, BL + b), (BL + b, b))):
                    anc = mn.tile([128, 2, 20], F32R, name=f"an_{d}_{b}_{ri}",
                                  tag="anc")
                    for c in range(2):
                        nc.vector.tensor_scalar_mul(
                            anc[:, c, :], w2T[:, c, 4 * di + FULL, :],
                            ancv[:, c, s_an:s_an + 1])
                    nps = mnp.tile([20, 64], F32, name=f"nf_{d}_{b}_{ri}",
                                   tag="nf")
                    for c in range(2):
                        nc.tensor.matmul(nps[:], anc[:, c, :],
                                         cT[:, c, s_me, :],
                                         start=(c == 0), stop=(c == 1))
                    den = mn.tile([20, 64], F32, name=f"de_{d}_{b}_{ri}",
                                  tag="den")
                    nc.vector.tensor_scalar(
                        out=den[:], in0=n1[:, FULL, s_me, :],
                        scalar1=n1[:, FULL, s_an, anchor_t:anchor_t + 1],
                        scalar2=EPS, op0=ALU.mult, op1=ALU.max)
                    nc.vector.reciprocal(den[:], den[:])
                    nc.vector.tensor_tensor(out=mcol(mt, 0, ri, b), in0=nps[:],
                                            in1=den[:], op=ALU.mult)

        # ---- MAX (pair): scaled builds
        with tc.tile_pool(name=f"mx_{d}", bufs=1) as mxp, \
             tc.tile_pool(name=f"mxps_{d}", bufs=1, space="PSUM") as mxps:
            # DRAM staging of clamped-recip norms (MAX type) per role
            nbc = {}
            for ri, s0 in ((0, 0), (1, BL)):
                dscr = dramp.tile([20, 256], F32, name=f"nd_{d}_{ri}")
                nc.sync.dma_start(
                    dscr[:], rn1[:, MAX, s0:s0 + BL, :].rearrange(
                        "l s t -> l (s t)"))
                nbc[ri] = dscr
            for bp in range(2):
                scp = mxp.tile([128, 20, 2, 2, 64], BF16,
                               name=f"scp_{d}_{bp}", tag="scp")
                sch = mxp.tile([128, 20, 2, 2, 64], BF16,
                               name=f"sch_{d}_{bp}", tag="sch")
                for l in range(L):
                    for c in range(2):
                        e1 = nc.vector if (l + c) % 2 == 0 else nc.gpsimd
                        e2 = nc.gpsimd if (l + c) % 2 == 0 else nc.vector
                        e1.tensor_scalar_mul(
                            scp[:, l, c, :, :],
                            cbf[:, c, 2 * bp:2 * bp + 2, :],
                            w2Tf[:, c, 4 * di + MAX, l:l + 1])
                        e2.tensor_scalar_mul(
                            sch[:, l, c, :, :],
                            cbf[:, c, BL + 2 * bp:BL + 2 * bp + 2, :],
                            w2Tf[:, c, 4 * di + MAX, l:l + 1])
                mxs_all = {}
                for side in range(2):
                    for b in (2 * bp, 2 * bp + 1):
                        mxs_all[(side, b)] = mxp.tile(
                            [64, 2, 10], F32, name=f"mxs_{d}_{b}_{side}",
                            tag=f"mxs_{side}_{b % 2}")
                for side in range(2):  # 0: max over j (p rows); 1: over i
                    sc_l = scp if side == 0 else sch
                    rs0 = BL if side == 0 else 0
                    for hf in range(2):
                        pps = mxps.tile([128, 10, 128], F32,
                                        name=f"pp_{d}_{bp}_{side}_{hf}",
                                        tag="pps", bufs=2)
                        for u in range(10):
                            l = 10 * hf + u
                            for c in range(2):
                                nc.tensor.matmul(
                                    pps[:, u, :],
                                    sc_l[:, l, c, :, :].rearrange(
                                        "k e t -> k (e t)"),
                                    cbf[:, c, rs0 + 2 * bp:rs0 + 2 * bp + 2,
                                        :].rearrange("k e t -> k (e t)"),
                                    start=(c == 0), stop=(c == 1))
                        for b in (2 * bp, 2 * bp + 1):
                            beta = b % 2
                            src_d = nbc[1] if side == 0 else nbc[0]
                            nbcb = mxp.tile([64, 10, 64], F32,
                                            name=f"nb_{d}_{b}_{side}_{hf}",
                                            tag="nbcb", bufs=2)
                            nc.sync.dma_start(
                                nbcb[:],
                                bass.AP(tensor=src_d.tensor,
                                        offset=src_d.offset
                                        + hf * 10 * 256 + b * 64,
                                        ap=[[0, 64], [256, 10], [1, 64]]))
                            pn = mxp.tile([64, 10, 64], BF16,
                                          name=f"pn_{d}_{b}_{side}_{hf}",
                                          tag="pn", bufs=2)
                            nc.vector.tensor_tensor(
                                out=pn[:],
                                in0=pps[64 * beta:64 * beta + 64, :,
                                        64 * beta:64 * beta + 64],
                                in1=nbcb[:], op=ALU.mult)
                            nc.vector.tensor_reduce(
                                out=mxs_all[(side, b)][:, hf, :], in_=pn[:],
                                axis=AX_X, op=ALU.max)
                for side in range(2):
                    for b in (2 * bp, 2 * bp + 1):
                        yt = mxps.tile([20, 64], F32,
                                       name=f"yt_{d}_{b}_{side}", tag="yt",
                                       bufs=2)
                        nc.tensor.transpose(
                            yt[:],
                            mxs_all[(side, b)].rearrange(
                                "t hf u -> t (hf u)"),
                            idf[0:64, 0:64])
                        ri_me = 0 if side == 0 else 1
                        s_me = b if side == 0 else BL + b
                        nc.vector.tensor_tensor(
                            out=mcol(mt, 32, ri_me, b), in0=yt[:],
                            in1=rn1[:, MAX, s_me, :], op=ALU.mult)

        # ---- AM + AX per (b)
        with tc.tile_pool(name=f"am_{d}", bufs=2) as amp, \
             tc.tile_pool(name=f"amps_{d}", bufs=2, space="PSUM") as amps:
            for b in range(BL):
                _am_ax_block(nc, tc, d, di, b, cT, cbf, csq, w2T, n1, rn1,
                             rvn, ctm, mvT[di], idf, amp, amps, dramp, mcol,
                             anchor_t)


def _am_ax_block(nc, tc, d, di, b, cT, cbf, csq, w2T, n1, rn1, rvn, ctm,
                 mt, idf, amp, amps, dramp, mcol, anchor_t):
    AM_SLOT, AX_SLOT = 64, 96
    AMTY, AXTY = 2, 3
    sp, sh = b, 4 + b

    # raw attention + transpose
    att_ps = amps.tile([64, 64], F32, name=f"at_{d}_{b}", tag="t64",
                       bufs=3)
    for c in range(2):
        nc.tensor.matmul(att_ps[:], cT[:, c, sp, :], cT[:, c, sh, :],
                         start=(c == 0), stop=(c == 1))
    # normalize: att_norm = rvn_p[i] * att * rvn_h[j]
    a1 = amp.tile([64, 64], F32, name=f"a1_{d}_{b}", tag="a1")
    nc.vector.tensor_scalar_mul(a1[:], att_ps[:], rvn[:, sp:sp + 1])
    a1t_ps = amps.tile([64, 64], F32, name=f"a1t_{d}_{b}", tag="t64", bufs=3)
    nc.tensor.transpose(a1t_ps[:], a1[:], idf[0:64, 0:64])
    attTn = amp.tile([64, 64], F32, name=f"aTn_{d}_{b}", tag="attTn")
    nc.vector.tensor_scalar_mul(attTn[:], a1t_ps[:], rvn[:, sh:sh + 1])
    attn_ps = amps.tile([64, 64], F32, name=f"an2_{d}_{b}", tag="t64", bufs=3)
    nc.tensor.transpose(attn_ps[:], attTn[:], idf[0:64, 0:64])
    attn = amp.tile([64, 64], F32, name=f"an_{d}_{b}", tag="attn")
    nc.scalar.activation(attn[:], attn_ps[:], AF.Copy)

    # row sums + clamped recips
    rs_h = amp.tile([64, 1], F32, name=f"rh_{d}_{b}", tag="rsh")
    nc.vector.tensor_reduce(out=rs_h[:], in_=attn[:], axis=AX_X, op=ALU.add)
    nc.vector.tensor_scalar_max(rs_h[:], rs_h[:], EPS)
    nc.vector.reciprocal(rs_h[:], rs_h[:])
    rs_p = amp.tile([64, 1], F32, name=f"rp_{d}_{b}", tag="rsp")
    nc.vector.tensor_reduce(out=rs_p[:], in_=attTn[:], axis=AX_X, op=ALU.add)
    nc.vector.tensor_scalar_max(rs_p[:], rs_p[:], EPS)
    nc.vector.reciprocal(rs_p[:], rs_p[:])

    # weighted mean rhs: AhT = T(attn * rs_h), BpT = T(attTn * rs_p)
    ah = amp.tile([64, 64], F32, name=f"ah_{d}_{b}", tag="ah")
    nc.vector.tensor_scalar_mul(ah[:], attn[:], rs_h[:, 0:1])
    ahT_ps = amps.tile([64, 64], F32, name=f"ahT_{d}_{b}", tag="t64", bufs=3)
    nc.tensor.transpose(ahT_ps[:], ah[:], idf[0:64, 0:64])
    ahT = amp.tile([64, 64], F32R, name=f"ahTs_{d}_{b}", tag="ahTs")
    nc.scalar.activation(ahT[:], ahT_ps[:], AF.Copy)
    bp = amp.tile([64, 64], F32, name=f"bp_{d}_{b}", tag="bp")
    nc.vector.tensor_scalar_mul(bp[:], attTn[:], rs_p[:, 0:1])
    bpT_ps = amps.tile([64, 64], F32, name=f"bpT_{d}_{b}", tag="t64", bufs=3)
    nc.tensor.transpose(bpT_ps[:], bp[:], idf[0:64, 0:64])
    bpT = amp.tile([64, 64], F32R, name=f"bpTs_{d}_{b}", tag="bpTs")
    nc.scalar.activation(bpT[:], bpT_ps[:], AF.Copy)

    # am vectors (hd-major): am_hT[hc][h,i], am_pT[hc][h,j]
    for role, (rhs, s_ctm, s_me, coln) in enumerate(
            ((ahT, sh, sp, 0), (bpT, sp, sh, 1))):
        amv_ps = amps.tile([128, 2, 64], F32, name=f"av_{d}_{b}_{role}",
                           tag="amv", bufs=1)
        for c in range(2):
            nc.tensor.matmul(amv_ps[:, c, :],
                             ctm[:, s_ctm, 128 * c:128 * (c + 1)], rhs[:],
                             start=True, stop=True)
        amv = amp.tile([128, 2, 64], F32R, name=f"am_{d}_{b}_{role}",
                       tag="amv_sb")
        nc.scalar.activation(amv.rearrange("k c t -> k (c t)"),
                             amv_ps.rearrange("k c t -> k (c t)"), AF.Copy)
        # num = W2 @ (v * am)
        prod = amp.tile([128, 2, 64], F32R, name=f"pr_{d}_{b}_{role}",
                        tag="prod")
        for c in range(2):
            nc.vector.tensor_tensor(out=prod[:, c, :], in0=cT[:, c, s_me, :],
                                    in1=amv[:, c, :], op=ALU.mult)
        nump = amps.tile([20, 64], F32, name=f"nu_{d}_{b}_{role}", tag="s20", bufs=2)
        for c in range(2):
            nc.tensor.matmul(nump[:], w2T[:, c, 4 * di + AMTY, :],
                             prod[:, c, :], start=(c == 0), stop=(c == 1))
        # n2 = sqrt(W2 @ am^2)
        amsq = amp.tile([128, 2, 64], F32R, name=f"as_{d}_{b}_{role}",
                        tag="amsq")
        nc.scalar.activation(amsq.rearrange("k c t -> k (c t)"),
                             amv.rearrange("k c t -> k (c t)"), AF.Square)
        n2p = amps.tile([20, 64], F32, name=f"n2_{d}_{b}_{role}", tag="s20", bufs=2)
        for c in range(2):
            nc.tensor.matmul(n2p[:], w2T[:, c, 4 * di + AMTY, :],
                             amsq[:, c, :], start=(c == 0), stop=(c == 1))
        n2s = amp.tile([20, 64], F32, name=f"ns_{d}_{b}_{role}", tag="n2s")
        nc.scalar.activation(n2s[:], n2p[:], AF.Sqrt)
        den = amp.tile([20, 64], F32, name=f"dn_{d}_{b}_{role}", tag="amden")
        nc.vector.tensor_tensor(out=den[:], in0=n1[:, AMTY, s_me, :],
                                in1=n2s[:], op=ALU.mult)
        nc.vector.tensor_scalar_max(den[:], den[:], EPS)
        nc.vector.reciprocal(den[:], den[:])
        nc.vector.tensor_tensor(out=mcol(mt, AM_SLOT, role, b), in0=nump[:],
                                in1=den[:], op=ALU.mult)

    # ---- AX: att_h.max / att_p.max via DRAM-broadcast of attn
    atb = amp.tile([64, 64], BF16, name=f"ab_{d}_{b}", tag="atb")
    nc.vector.tensor_copy(atb[:], attn[:])
    atbT = amp.tile([64, 64], BF16, name=f"abT_{d}_{b}", tag="atbT")
    nc.vector.tensor_copy(atbT[:], attTn[:])
    dsc = dramp.tile([64, 64], BF16, name=f"dx_{d}_{b}")
    nc.sync.dma_start(dsc[:], atb[:])
    dscT = dramp.tile([64, 64], BF16, name=f"dxT_{d}_{b}")
    nc.sync.dma_start(dscT[:], atbT[:])
    bch = amp.tile([128, 64, 64], BF16, name=f"bc_{d}_{b}", tag="bch", bufs=1)
    nc.sync.dma_start(bch[:], bass.AP(tensor=dsc.tensor, offset=dsc.offset,
                                      ap=[[0, 128], [64, 64], [1, 64]]))
    bcp = amp.tile([128, 64, 64], BF16, name=f"bcT_{d}_{b}", tag="bcp", bufs=1)
    nc.sync.dma_start(bcp[:], bass.AP(tensor=dscT.tensor, offset=dscT.offset,
                                      ap=[[0, 128], [64, 64], [1, 64]]))

    for role in range(2):
        s_v = sh if role == 0 else sp      # the "other" sequence vectors
        s_me = sp if role == 0 else sh
        bc = bch if role == 0 else bcp
        axm = amp.tile([128, 2, 64], F32R, name=f"axm_{d}_{b}_{role}",
                       tag="axm")
        for c in range(2):
            eng = nc.vector if (b + role + c) % 2 == 0 else nc.gpsimd
            prod = amp.tile([128, 64, 64], BF16,
                            name=f"xp_{d}_{b}_{role}_{c}", tag="xprod", bufs=1)
            vb = cbf[:, c, s_v, :]
            eng.tensor_tensor(
                out=prod[:],
                in0=bass.AP(tensor=vb.tensor, offset=vb.offset,
                            ap=[vb.ap[0], [0, 64], vb.ap[1]]),
                in1=bc[:], op=ALU.mult)
            nc.vector.tensor_reduce(out=axm[:, c, :], in_=prod[:],
                                    axis=AX_X, op=ALU.max)
        # cos(vp, axm) under w_ax
        prodx = amp.tile([128, 2, 64], F32R, name=f"px_{d}_{b}_{role}",
                         tag="prodx")
        for c in range(2):
            nc.vector.tensor_tensor(out=prodx[:, c, :], in0=cT[:, c, s_me, :],
                                    in1=axm[:, c, :], op=ALU.mult)
        nux = amps.tile([20, 64], F32, name=f"nx_{d}_{b}_{role}", tag="s20", bufs=2)
        for c in range(2):
            nc.tensor.matmul(nux[:], w2T[:, c, 4 * di + AXTY, :],
                             prodx[:, c, :], start=(c == 0), stop=(c == 1))
        axsq = amp.tile([128, 2, 64], F32R, name=f"xs_{d}_{b}_{role}",
                        tag="axsq")
        nc.scalar.activation(axsq.rearrange("k c t -> k (c t)"),
                             axm.rearrange("k c t -> k (c t)"), AF.Square)
        n2x = amps.tile([20, 64], F32, name=f"n2x_{d}_{b}_{role}", tag="s20", bufs=2)
        for c in range(2):
            nc.tensor.matmul(n2x[:], w2T[:, c, 4 * di + AXTY, :],
                             axsq[:, c, :], start=(c == 0), stop=(c == 1))
        n2xs = amp.tile([20, 64], F32, name=f"nxs_{d}_{b}_{role}", tag="n2xs")
        nc.scalar.activation(n2xs[:], n2x[:], AF.Sqrt)
        den = amp.tile([20, 64], F32, name=f"dx2_{d}_{b}_{role}", tag="axden")
        nc.vector.tensor_tensor(out=den[:], in0=n1[:, AXTY, s_me, :],
                                in1=n2xs[:], op=ALU.mult)
        nc.vector.tensor_scalar_max(den[:], den[:], EPS)
        nc.vector.reciprocal(den[:], den[:])
        nc.vector.tensor_tensor(out=mcol(mt, AX_SLOT, role, b), in0=nux[:],
                                in1=den[:], op=ALU.mult)


# ---------------------------------------------------------------- entry

def _get_nc(debug=False):
    key = ('dbg' if debug else 'rel')
    if key not in _CACHE:
        _CACHE[key] = build_nc(debug)
    return _CACHE[key]


def kernel(**inputs):
    nc = _get_nc(False)
    w = _prep_weights(inputs)
    in_maps = []
    for core in range(NCORES):
        m = dict(w)
        m['tokp'] = _prep_tokens(inputs['q1_inputs'], inputs['q2_inputs'],
                                 core)
        in_maps.append(m)
    res = run_bass_kernel_spmd(nc, in_maps, core_ids=list(range(NCORES)))
    out = np.concatenate([res.results[c]['y'] for c in range(NCORES)], axis=0)
    return out.astype(np.float32)


def run_debug(inputs):
    nc = _get_nc(True)
    w = _prep_weights(inputs)
    in_maps = []
    for core in range(NCORES):
        m = dict(w)
        m['tokp'] = _prep_tokens(inputs['q1_inputs'], inputs['q2_inputs'],
                                 core)
        in_maps.append(m)
    res = run_bass_kernel_spmd(nc, in_maps, core_ids=list(range(NCORES)))
    return res
